# revision 1
# baseline (speedup 1.0000x reference)
"""Complex multi-head attention on 8 Trainium2 NeuronCores.

Sharding: core c handles batch b = c//2 and heads 4*(c%2) .. 4*(c%2)+4
(tensor-parallel over the 8 heads within each batch pair). Each core
computes its 4 heads end-to-end (QKV projections restricted to its head
columns, attention, and its partial contribution to the output
projection). The pairwise partial-sum reduction of the output projection
is done on the host during unshard (plus all bias constants that commute
through the softmax).

Math notes:
  - complex linear: real = xr@wr.T - xi@wi.T + (br-bi),
                    imag = xi@wr.T + xr@wi.T + (br+bi)
  - scores (pre-softmax) are computed transposed [k, q] with the r/i
    feature dims stacked so the contraction is a full 128 partitions:
       sr^T = [kr;ki]^T.T @ [qr;qi]^T,  si^T = [-ki;kr]^T.T @ [qr;qi]^T
  - softmax argument is |s| = SCALE*sqrt(sr^2+si^2); exp(|s|) never
    overflows for this data scale so the max-subtraction is skipped.
    |s| is evaluated as exp(0.5*ln(msq) + ln(SCALE)) so ln/exp stay in
    one ACT table set (no table reloads).
  - V biases and output-projection biases commute through the softmax
    (attn rows sum to 1) and are added on the host.
"""

import numpy as np

B, S, E, H = 4, 1024, 512, 8
HD = E // H  # 64
SCALE = HD ** -0.5
N_CORES = 8
HPC = H // 2          # heads per core = 4
FPC = HPC * HD        # feature cols per core = 256
EC = E // 128         # contraction chunks = 4
QB = S // 512         # 512-wide q blocks = 2
KC = S // 128         # 128-wide k chunks = 8
SC = S // 128         # s chunks for V = 8

_CACHE = {}


def _patch_tile_drain():
    """The tile-exit drain attaches one sem wait per live logical
    processor; this walrus build accepts a single sync wait per CTRL
    instruction. Split the waits across a chain of drains on the same
    engine (program order preserves the semantics)."""
    import concourse.tile as tile_mod
    from concourse.vector_clock import ScopedClock

    if getattr(tile_mod.TileContext, "_drain_split_patched", False):
        return

    def _patched(self, tick_clock, wait_clock):
        nc = self.nc
        drain_inst = nc.sync.drain()
        wait_clock.add_sem_waits(
            drain_inst.ins, ScopedClock({None: tick_clock.global_clock})
        )
        si = drain_inst.ins.sync_info
        waits = list(si.on_wait) if si and si.on_wait else []
        if len(waits) > 1:
            si.on_wait = waits[:1]
            for w in waits[1:]:
                extra = nc.sync.drain()
                esi = extra.ins.sync_info
                if esi is None:
                    import concourse.mybir as mybir
                    extra.ins.sync_info = mybir.SyncInfo(on_wait=[w], on_update=[])
                else:
                    esi.on_wait = list(esi.on_wait or []) + [w]
        nc.all_engine_barrier()
        assert self.sems is not None
        popped = nc._tile_sem_poison_stack.pop()
        assert popped is self._sem_poison
        nc.clear_and_free_semaphores(list(self.sems.allocated().values()))
        nc.all_engine_barrier()

    tile_mod.TileContext._drain_and_barrier = _patched
    tile_mod.TileContext._drain_split_patched = True


def _split_multi_waits(nc):
    """This walrus build accepts a single sync wait per instruction.
    Hoist extra waits onto same-engine NOPs inserted just before the
    instruction (waits execute earlier on the same engine: semantics
    preserved, strictly more conservative)."""
    import concourse.mybir as mybir

    ctr = [0]
    for f in nc.m.functions:
        for bb in f.blocks:
            out = []
            changed = False
            for ins in bb.instructions:
                si = ins.sync_info
                waits = list(si.on_wait) if si and si.on_wait else []
                if len(waits) > 1:
                    changed = True
                    for w in waits[:-1]:
                        ctr[0] += 1
                        nop = mybir.InstNoOp(
                            name=f"W-split-{ctr[0]}",
                            sync_info=mybir.SyncInfo(on_wait=[w], on_update=[]),
                            engine=ins.engine,
                            bass_nofuse=True,
                        )
                        out.append(nop)
                    si.on_wait = waits[-1:]
                out.append(ins)
            if changed:
                bb.instructions = out


def _build_program(reps=1):
    import os as _os
    GP_ADD = _os.environ.get("KV", "gp") == "gp"
    STOP = _os.environ.get("STOP", "full")  # qkv | scores | noz | nonorm | full
    import concourse.bass as bass
    from concourse import mybir
    from concourse.tile import TileContext

    _patch_tile_drain()

    f32 = mybir.dt.float32
    f32r = mybir.dt.float32r
    LN_SCALE = float(np.log(SCALE))

    nc = bass.Bass()
    dp = nc.declare_dram_parameter
    xrT = dp("xrT", [E, S], f32r, isOutput=False)
    xiT = dp("xiT", [E, S], f32r, isOutput=False)
    wqa = dp("wqa", [E, FPC * 2], f32r, isOutput=False)
    wqb = dp("wqb", [E, FPC * 2], f32r, isOutput=False)
    wka = dp("wka", [E, FPC * 2], f32r, isOutput=False)
    wkb = dp("wkb", [E, FPC * 2], f32r, isOutput=False)
    wva = dp("wva", [E, FPC * 2], f32r, isOutput=False)
    wvb = dp("wvb", [E, FPC * 2], f32r, isOutput=False)
    bq = dp("bq", [128, HPC], f32, isOutput=False)
    bk = dp("bk", [128, HPC], f32, isOutput=False)
    wor = dp("wor", [HPC, 128, E], f32r, isOutput=False)
    ones_d = dp("ones", [128, 1], f32r, isOutput=False)
    woi = dp("woi", [HPC, 128, E], f32r, isOutput=False)
    yrT = dp("yrT", [E, S], f32, isOutput=True)
    yiT = dp("yiT", [E, S], f32, isOutput=True)

    with TileContext(nc) as tc:
        from contextlib import ExitStack

        for _rep in range(reps):
          with ExitStack() as outer:
            consts = outer.enter_context(tc.tile_pool(name="consts", bufs=1))
            qkv_out = outer.enter_context(tc.tile_pool(name="qkv", bufs=1))

            ones_sb = consts.tile([128, 1], f32r)
            nc.sync.dma_start(out=ones_sb, in_=ones_d[:, :])
            lnscale_sb = consts.tile([128, 1], f32)
            nc.vector.memset(lnscale_sb, LN_SCALE)
            bq_sb = consts.tile([128, HPC], f32)
            nc.sync.dma_start(out=bq_sb, in_=bq[:, :])
            bk_sb = consts.tile([128, HPC], f32)
            nc.sync.dma_start(out=bk_sb, in_=bk[:, :])

            # persistent attention operands
            Q_sb = [qkv_out.tile([128, S], f32r, tag=f"Q{h}", name=f"Q{h}") for h in range(HPC)]
            K_sb = [qkv_out.tile([128, S], f32r, tag=f"K{h}", name=f"K{h}") for h in range(HPC)]
            K2_sb = [qkv_out.tile([128, S], f32r, tag=f"K2{h}", name=f"K2{h}") for h in range(HPC)]
            V_sb = [qkv_out.tile([128, FPC * 2], f32r, tag=f"V{sc}", name=f"V{sc}") for sc in range(SC)]

            # ---------------- QKV projections ----------------
            avs = outer.enter_context(tc.tile_pool(name="avs", bufs=1))
            wo_pool = outer.enter_context(tc.tile_pool(name="wo", bufs=1))
            yout = outer.enter_context(tc.tile_pool(name="yout", bufs=2))

            with ExitStack() as qkv_ctx:
                xw = qkv_ctx.enter_context(tc.tile_pool(name="xw", bufs=1))
                psum_p = qkv_ctx.enter_context(
                    tc.tile_pool(name="psum_p", bufs=2, space="PSUM")
                )

                xr_sb = [xw.tile([128, S], f32r, tag=f"xr{ec}", name=f"xr{ec}") for ec in range(EC)]
                xi_sb = [xw.tile([128, S], f32r, tag=f"xi{ec}", name=f"xi{ec}") for ec in range(EC)]
                for ec in range(EC):
                    nc.sync.dma_start(out=xr_sb[ec], in_=xrT[128 * ec:128 * ec + 128, :])
                    nc.sync.dma_start(out=xi_sb[ec], in_=xiT[128 * ec:128 * ec + 128, :])

                w_sb = {}
                for name, dram in (("qa", wqa), ("qb", wqb), ("ka", wka),
                                   ("kb", wkb), ("va", wva), ("vb", wvb)):
                    tiles = []
                    for ec in range(EC):
                        t = xw.tile([128, FPC * 2], f32r, tag=f"w{name}{ec}", name=f"w{name}{ec}")
                        nc.sync.dma_start(out=t, in_=dram[128 * ec:128 * ec + 128, :])
                        tiles.append(t)
                    w_sb[name] = tiles

                # Q and K (transposed layout, bias per partition)
                for h in range(HPC):
                    cs = slice(128 * h, 128 * h + 128)
                    for dst, wa, wb, bias in (
                        (Q_sb[h], w_sb["qa"], w_sb["qb"], bq_sb),
                        (K_sb[h], w_sb["ka"], w_sb["kb"], bk_sb),
                    ):
                        ps = psum_p.tile([128, S], f32, tag="proj_ps", name="proj_ps")
                        for qb in range(QB):
                            qs = slice(512 * qb, 512 * qb + 512)
                            for i, ec in enumerate(range(EC)):
                                nc.tensor.matmul(
                                    ps[:, qs], wa[ec][:, cs], xr_sb[ec][:, qs],
                                    start=(i == 0), stop=False)
                            for i, ec in enumerate(range(EC)):
                                nc.tensor.matmul(
                                    ps[:, qs], wb[ec][:, cs], xi_sb[ec][:, qs],
                                    start=False, stop=(i == EC - 1))
                        nc.vector.tensor_scalar_add(dst, ps, bias[:, h:h + 1])
                    # K2 = [-ki; kr] from K (biases already included)
                    nc.vector.tensor_scalar_mul(K2_sb[h][0:64, :], K_sb[h][64:128, :], -1.0)
                    nc.vector.tensor_scalar_mul(K2_sb[h][64:128, :], K_sb[h][0:64, :], 1.0)

                # V natural layout (no bias; folded into host constants)
                for sc in range(SC):
                    ss = slice(128 * sc, 128 * sc + 128)
                    ps = psum_p.tile([128, FPC * 2], f32, tag="v_ps", name="v_ps")
                    for i, ec in enumerate(range(EC)):
                        nc.tensor.matmul(ps, xr_sb[ec][:, ss], w_sb["va"][ec],
                                         start=(i == 0), stop=False)
                    for i, ec in enumerate(range(EC)):
                        nc.tensor.matmul(ps, xi_sb[ec][:, ss], w_sb["vb"][ec],
                                         start=False, stop=(i == EC - 1))
                    nc.scalar.copy(V_sb[sc], ps)

            # ---------------- attention + output projection ----------------
            with ExitStack() as att_ctx:
                Exp = mybir.ActivationFunctionType.Exp
                Ln = mybir.ActivationFunctionType.Ln
                Square = mybir.ActivationFunctionType.Square
                sc_pool = att_ctx.enter_context(
                    tc.tile_pool(name="sc_ps", bufs=2, space="PSUM"))
                av_pool = att_ctx.enter_context(
                    tc.tile_pool(name="av_ps", bufs=1, space="PSUM"))
                z_pool = att_ctx.enter_context(
                    tc.tile_pool(name="z_ps", bufs=1, space="PSUM"))
                ew = att_ctx.enter_context(tc.tile_pool(name="ew", bufs=2))
                epool = att_ctx.enter_context(tc.tile_pool(name="epool", bufs=KC + 1))
                zdram = att_ctx.enter_context(
                    tc.tile_pool(name="zdram", bufs=2, space="DRAM"))

                wor_sb = []
                woi_sb = []
                for h in range(HPC):
                    t = wo_pool.tile([128, E], f32r, tag=f"wor{h}", name=f"wor{h}")
                    nc.sync.dma_start(out=t, in_=wor[h, :, :])
                    wor_sb.append(t)
                    t = wo_pool.tile([128, E], f32r, tag=f"woi{h}", name=f"woi{h}")
                    nc.sync.dma_start(out=t, in_=woi[h, :, :])
                    woi_sb.append(t)

                av_sb = [avs.tile([128, S], f32r, tag=f"av{h}", name=f"av{h}") for h in range(HPC)]

                for h in range(HPC):
                    if STOP == "qkv":
                        break
                    e_tiles = []
                    av_ps = [av_pool.tile([128, 512], f32, tag=f"avp{qb}", name=f"avp{qb}")
                             for qb in range(QB)]
                    z_ps = [z_pool.tile([1, 512], f32, tag=f"zp{qb}", name=f"zp{qb}")
                            for qb in range(QB)]
                    for kc in range(KC):
                        ks = slice(128 * kc, 128 * kc + 128)
                        u12 = ew.tile([128, S], f32, tag="u12", name="u12")
                        w12 = ew.tile([128, S], f32, tag="w12", name="w12")
                        for qb in range(QB):
                            qs = slice(512 * qb, 512 * qb + 512)
                            sr_ps = sc_pool.tile([128, 512], f32, tag="sr", name="sr")
                            nc.tensor.matmul(sr_ps, K_sb[h][:, ks], Q_sb[h][:, qs],
                                             start=True, stop=True)
                            si_ps = sc_pool.tile([128, 512], f32, tag="si", name="si")
                            nc.tensor.matmul(si_ps, K2_sb[h][:, ks], Q_sb[h][:, qs],
                                             start=True, stop=True)
                            nc.scalar.activation(u12[:, qs], sr_ps, Square)
                            nc.scalar.activation(w12[:, qs], si_ps, Square)
                        msq = ew.tile([128, S], f32, tag="msq", name="msq")
                        if GP_ADD:
                            nc.gpsimd.tensor_tensor(msq, u12, w12, mybir.AluOpType.add)
                        else:
                            nc.vector.tensor_add(msq, u12, w12)
                        # reuse u12/w12 (dead) for the ln/exp chain
                        nc.scalar.activation(u12, msq, Ln)
                        nc.scalar.activation(w12, u12, Exp, bias=lnscale_sb, scale=0.5)
                        e_t = epool.tile([128, S], f32r, tag="e", name="e")
                        nc.scalar.activation(e_t, w12, Exp)
                        e_tiles.append(e_t)

                        for qb in range(QB):
                            qs = slice(512 * qb, 512 * qb + 512)
                            nc.tensor.matmul(
                                av_ps[qb], V_sb[kc][:, 128 * h:128 * h + 128],
                                e_t[:, qs], start=(kc == 0), stop=(kc == KC - 1))
                            if STOP != "noz":
                                nc.tensor.matmul(
                                    z_ps[qb], ones_sb, e_t[:, qs],
                                    start=(kc == 0), stop=(kc == KC - 1))

                    if STOP in ("noz", "nonorm"):
                        for qb in range(QB):
                            qs = slice(512 * qb, 512 * qb + 512)
                            nc.vector.tensor_copy(av_sb[h][:, qs], av_ps[qb])
                        continue
                    # normalize: av * (1/Z) broadcast across partitions
                    # (broadcast via a DRAM round-trip: SBUF broadcast APs are
                    # rejected, DRAM partition-stride-0 reads are fine)
                    zr = ew.tile([1, S], f32, tag="zr", name="zr")
                    for qb in range(QB):
                        nc.vector.reciprocal(
                            zr[:, 512 * qb:512 * qb + 512], z_ps[qb])
                    zd = zdram.tile([1, S], f32, tag="zd", name="zd")
                    nc.sync.dma_start(out=zd, in_=zr)
                    zb = ew.tile([128, S], f32, tag="zb", name="zb")
                    zd_b = bass.AP(
                        tensor=zd.tensor, offset=zd.offset,
                        ap=[[0, 128]] + list(zd.ap[1:]))
                    nc.sync.dma_start(out=zb, in_=zd_b)
                    for qb in range(QB):
                        qs = slice(512 * qb, 512 * qb + 512)
                        nc.vector.tensor_mul(av_sb[h][:, qs], av_ps[qb], zb[:, qs])

            # output projection: yrT/yiT [E, S]
            with ExitStack() as yctx:
                y_psum = yctx.enter_context(
                    tc.tile_pool(name="y_ps", bufs=2, space="PSUM"))
                for dst, w in (((yrT, wor_sb), (yiT, woi_sb)) if STOP != "qkv" else ()):
                    for eco in range(EC):
                        es = slice(128 * eco, 128 * eco + 128)
                        ps = y_psum.tile([128, S], f32, tag="y", name="y")
                        for qb in range(QB):
                            qs = slice(512 * qb, 512 * qb + 512)
                            for h in range(HPC):
                                nc.tensor.matmul(
                                    ps[:, qs], w[h][:, es], av_sb[h][:, qs],
                                    start=(h == 0), stop=(h == HPC - 1))
                        yt = yout.tile([128, S], f32, tag="yt", name="yt")
                        nc.scalar.copy(yt, ps)
                        nc.sync.dma_start(out=dst[es, :], in_=yt)

    _split_multi_waits(nc)
    return nc


def _prep_core_inputs(inputs, c):
    f32 = np.float32
    b, j = c // 2, c % 2
    hs = slice(FPC * j, FPC * j + FPC)

    def stacks(wr, wi):
        # A (applied to xr^T) and B (applied to xi^T): per head h the
        # 128-col block is [wr[fs].T | wi[fs].T] resp. [-wi[fs].T | wr[fs].T]
        A = np.empty((E, FPC * 2), f32)
        Bm = np.empty((E, FPC * 2), f32)
        for h in range(HPC):
            fs = slice(hs.start + HD * h, hs.start + HD * h + HD)
            A[:, 128 * h:128 * h + 64] = wr[fs, :].T
            A[:, 128 * h + 64:128 * h + 128] = wi[fs, :].T
            Bm[:, 128 * h:128 * h + 64] = -wi[fs, :].T
            Bm[:, 128 * h + 64:128 * h + 128] = wr[fs, :].T
        return np.ascontiguousarray(A), np.ascontiguousarray(Bm)

    wqa, wqb = stacks(inputs["q_wr"], inputs["q_wi"])
    wka, wkb = stacks(inputs["k_wr"], inputs["k_wi"])
    wva, wvb = stacks(inputs["v_wr"], inputs["v_wi"])

    def bias_cols(br, bi):
        out = np.empty((128, HPC), f32)
        for h in range(HPC):
            fs = slice(hs.start + HD * h, hs.start + HD * h + HD)
            out[0:64, h] = br[fs] - bi[fs]
            out[64:128, h] = br[fs] + bi[fs]
        return out

    bq = bias_cols(inputs["q_br"], inputs["q_bi"])
    bk = bias_cols(inputs["k_br"], inputs["k_bi"])

    o_wr, o_wi = inputs["o_wr"], inputs["o_wi"]
    wor = np.empty((HPC, 128, E), f32)
    woi = np.empty((HPC, 128, E), f32)
    for h in range(HPC):
        fs = slice(hs.start + HD * h, hs.start + HD * h + HD)
        wor[h, 0:64, :] = o_wr[:, fs].T
        wor[h, 64:128, :] = -o_wi[:, fs].T
        woi[h, 0:64, :] = o_wi[:, fs].T
        woi[h, 64:128, :] = o_wr[:, fs].T

    return {
        "xrT": np.ascontiguousarray(inputs["x_real"][b].T.astype(f32)),
        "xiT": np.ascontiguousarray(inputs["x_imag"][b].T.astype(f32)),
        "wqa": wqa, "wqb": wqb, "wka": wka, "wkb": wkb, "wva": wva, "wvb": wvb,
        "bq": bq, "bk": bk,
        "wor": np.ascontiguousarray(wor), "woi": np.ascontiguousarray(woi),
        "ones": np.ones((128, 1), f32),
    }


def kernel(**inputs):
    from concourse.bass_utils import run_bass_kernel_spmd

    if "nc" not in _CACHE:
        _CACHE["nc"] = _build_program()
    nc = _CACHE["nc"]

    core_ids = list(range(N_CORES))
    in_maps = [_prep_core_inputs(inputs, c) for c in core_ids]
    res = run_bass_kernel_spmd(nc, in_maps, core_ids)

    # host-side unshard: sum partial yT pairs, add bias constants, transpose
    f32 = np.float32
    o_wr, o_wi = inputs["o_wr"], inputs["o_wi"]
    cvr = inputs["v_br"] - inputs["v_bi"]
    cvi = inputs["v_br"] + inputs["v_bi"]
    yr_const = (inputs["o_br"] - inputs["o_bi"]) + o_wr @ cvr - o_wi @ cvi
    yi_const = (inputs["o_br"] + inputs["o_bi"]) + o_wi @ cvr + o_wr @ cvi

    yr = np.empty((B, S, E), f32)
    yi = np.empty((B, S, E), f32)
    for b in range(B):
        r0, r1 = res.results[2 * b], res.results[2 * b + 1]
        yr[b] = (r0["yrT"] + r1["yrT"]).T + yr_const[None, :]
        yi[b] = (r0["yiT"] + r1["yiT"]).T + yi_const[None, :]
    return yr, yi

